# revision 47
# baseline (speedup 1.0000x reference)
"""Trainium2 Bass kernel for causal self-attention with T5 relative position bias.

Problem (hardcoded): B=4, T=2048, C=1024, H=16, D=64, NUM_BUCKETS=32, MAX_DISTANCE=128.
Sharding over 8 cores: core c -> (batch b=c//2, head-group hg=c%2 of 8 heads).
Each core computes qkv projection for its heads, causal attention, and a partial
output projection (its heads' rows of W_proj); host sums the two partials per batch.

Key structure:
  - Heads are processed in PAIRS (2m on partitions 0-63, 2m+1 on 64-127); the two
    K=64 S-matmuls of a pair row-tile the PE at (0,0)/(64,0).
  - AV is "flipped": P tiles [tk,128tq] stationary, v+ones [tk,65] moving ->
    out [tq,65]; softmax rowsum lands as a per-partition column so normalization
    is a DVE reciprocal + tensor_scalar multiply.
  - The T5 bias table is host-divided by exp(b31); far tiles skip the mask-mult;
    exp uses a 2-level AP that skips head-B's dead zone on diagonal tiles.
  - Pipeline shape (measured-bottleneck driven; ~258us vs 287us baseline):
    * Inputs are host-pre-arranged partition-major so DMA descriptors are
      >=2KB; loads are priority-ordered across the sync+scalar HW-DGE queues
      (m0 weight slices + x first-quarters first) so the first S matmul lands
      ~28us in; warm-up dummy matmuls keep the PE HAM clock at 2.4 GHz.
    * Within a pair, j (k-tile index) runs DESCENDING (diagonal first); the
      last j's AV + normalize + transposes are DEFERRED into the next pair's
      stream, right after its first S pair, so the PE queue never head-blocks
      on exp at pair boundaries and the ACT engine stays saturated.
    * Fillers run BEFORE each S pair: chunk0: rest of qkv0 + qkv1; chunk1:
      qkv2; chunk2: qkv3; chunk3: proj0-2 (the ACT-bound chunk absorbs proj).
      In chunk 3 pops are gated off the first iterations of each pair to dodge
      a false DMA-counter dependency on the freshly issued pair-tail
      transposes (the HWDGE completion semaphore counts in queue order).
    * yp stores go on the gpsimd SWDGE queue (sharing the sync queue with the
      xbar transposes produced wrong results on HW); the final chunk's stores
      use the scalar queue, free once the last exp is done.
    * Tail: the last pair's y is transposed on the PE (identity matmul), so
      proj(chunk3) has no DMA dependencies and overlaps the tail drain.
"""

import sys

sys.path.insert(0, "/opt/trn_rl_repo")

import math
from collections import deque

import numpy as np

import concourse.bacc as bacc
import concourse.bass as bass
import concourse.mybir as mybir
import concourse.tile as tile
from concourse import bass_utils


def _ensure_axon_hooks():
    """bass_utils imports antenv.axon_hooks when BASS_TRACE is set under axon;
    this image's antenv lacks that submodule. Provide an inert one so a stray
    trace env var degrades to a warning instead of crashing the run."""
    try:
        import antenv.axon_hooks  # noqa: F401
    except Exception:
        try:
            import types

            import antenv

            hooks = types.ModuleType("antenv.axon_hooks")
            hooks._hook = None
            hooks.set_axon_ntff_profile_hook = lambda h: setattr(hooks, "_hook", h)
            hooks.get_axon_ntff_profile_hook = lambda: hooks._hook
            sys.modules["antenv.axon_hooks"] = hooks
            antenv.axon_hooks = hooks
        except Exception:
            pass


_ensure_axon_hooks()

B, T, C = 4, 2048, 1024
H, D = 16, 64
NUM_BUCKETS, MAX_DISTANCE = 32, 128
HL = 8  # local heads per core
CL = HL * D  # 512 local channels
NCORES = 8
NPAIR = HL // 2  # 4 head pairs per core

FP16 = mybir.dt.float16
FP32 = mybir.dt.float32

NT = T // 512  # 4 tq chunks of 512
NK = T // 128  # 16 tk tiles of 128
KC = C // 128  # 8 contraction chunks for qkv
MC = CL // 128  # 4 m-chunks of local channels

# ea table geometry: slice start s = (tq0 - tk0) + 384; near tiles only, so
# s in {0,128,256,384,512} and max used column is s+511 = 1023.
EA_W = 1024


def _build_program():
    nc = bacc.Bacc(None, target_bir_lowering=False)

    # Inputs are host-pre-arranged partition-major ([128, kc, ...]) so every
    # DMA descriptor covers >=2KB contiguous per partition row.  wq/wk are
    # additionally m-major so the m=0 slices (the only ones the startup
    # q/k chains need) can be prioritized on the load queues.
    xh = nc.dram_tensor("xh", [128, KC, T], FP16, kind="ExternalInput")
    wq = nc.dram_tensor("wq", [128, MC, KC, 128], FP16, kind="ExternalInput")
    wk = nc.dram_tensor("wk", [128, MC, KC, 128], FP16, kind="ExternalInput")
    wv = nc.dram_tensor("wv", [128, KC, CL], FP16, kind="ExternalInput")
    wp = nc.dram_tensor("wp", [128, MC, C], FP16, kind="ExternalInput")
    bqk = nc.dram_tensor("bqk", [2, CL], FP32, kind="ExternalInput")
    bvr = nc.dram_tensor("bvr", [128, CL], FP32, kind="ExternalInput")
    # per-PAIR tables: [pair, head-in-pair, 128, EA_W], host-divided by exp(b31)
    wexp = nc.dram_tensor("wexp", [NPAIR, 2, 128, EA_W], FP16, kind="ExternalInput")
    ident = nc.dram_tensor("ident", [128, 128], FP16, kind="ExternalInput")
    yp = nc.dram_tensor("yp", [C, T], FP16, kind="ExternalOutput")

    with tile.TileContext(nc) as tc:
        with (
            tc.tile_pool(name="w", bufs=1) as wpool,
            tc.tile_pool(name="big", bufs=1) as bigpool,
            tc.tile_pool(name="ea", bufs=1) as eapool,
            tc.tile_pool(name="p2", bufs=6) as p2pool,
            tc.tile_pool(name="pm", bufs=6) as pmpool,
            tc.tile_pool(name="sm", bufs=4) as smpool,
            tc.tile_pool(name="yo", bufs=8) as yopool,
            tc.tile_pool(name="ps", bufs=2, space="PSUM") as ps,
            tc.tile_pool(name="pav", bufs=2, space="PSUM") as pavp,
            tc.tile_pool(name="misc", bufs=2, space="PSUM") as miscp,
        ):
            # ---- weights / constants ----
            wq_sb = wpool.tile([128, KC, CL], FP16)
            wk_sb = wpool.tile([128, KC, CL], FP16)
            wv_sb = wpool.tile([128, KC, CL], FP16)
            wp_sb = wpool.tile([128, MC, C], FP16)
            bq_sb = wpool.tile([128, MC], FP32)
            bk_sb = wpool.tile([128, MC], FP32)
            bv_sb = wpool.tile([128, CL], FP32)
            xt_sb = bigpool.tile([128, KC, T], FP16)
            ea_sb = [
                eapool.tile([128, 2, EA_W], FP16, name=f"ea{p}") for p in range(NPAIR)
            ]

            bqk_r = bqk.rearrange("b (m p) -> b p m", p=128)
            wexp_r = wexp.rearrange("pr h p w -> pr p h w")

            # Priority-ordered loads split over the two HW-DGE queues
            # (sync + scalar), most-critical bytes first: the m=0 q/k weight
            # slices, then x (split across both queues), then wv/ea0 (first
            # pair's AV + mask), then the remaining weight slices.
            id_sb = wpool.tile([128, 128], FP16)

            nc.sync.dma_start(out=bq_sb, in_=bqk_r[0])
            nc.sync.dma_start(out=bk_sb, in_=bqk_r[1])
            nc.sync.dma_start(out=wq_sb[:, :, 0:128], in_=wq[:, 0])
            for kc in range(4):
                nc.sync.dma_start(out=xt_sb[:, kc, 0:512], in_=xh[:, kc, 0:512])
            nc.sync.dma_start(out=wv_sb, in_=wv[:])
            nc.sync.dma_start(out=bv_sb, in_=bvr[:])
            for m in range(1, MC):
                nc.sync.dma_start(
                    out=wq_sb[:, :, m * 128 : (m + 1) * 128], in_=wq[:, m]
                )
            for kc in range(4):
                nc.sync.dma_start(out=xt_sb[:, kc, 512:1024], in_=xh[:, kc, 512:1024])
            for kc in range(KC):
                nc.sync.dma_start(
                    out=xt_sb[:, kc, 1024:2048], in_=xh[:, kc, 1024:2048]
                )
            nc.sync.dma_start(out=wp_sb, in_=wp[:])
            nc.sync.dma_start(out=id_sb, in_=ident[:])

            nc.scalar.dma_start(out=wk_sb[:, :, 0:128], in_=wk[:, 0])
            for kc in range(4, KC):
                nc.scalar.dma_start(out=xt_sb[:, kc, 0:512], in_=xh[:, kc, 0:512])
            for kc in range(4, KC):
                nc.scalar.dma_start(out=xt_sb[:, kc, 512:1024], in_=xh[:, kc, 512:1024])
            nc.scalar.dma_start(out=ea_sb[0], in_=wexp_r[0])
            for m in range(1, MC):
                nc.scalar.dma_start(
                    out=wk_sb[:, :, m * 128 : (m + 1) * 128], in_=wk[:, m]
                )
                nc.scalar.dma_start(out=ea_sb[m], in_=wexp_r[m])

            # ---- persistent activations ----
            qT_sb = bigpool.tile([128, MC, T], FP16)  # c' = m*128 + p
            kT_sb = bigpool.tile([128, MC, T], FP16)
            v_sb = bigpool.tile([128, NK, HL * 65], FP16)  # slot l: [v(64), ones]
            y_sb = bigpool.tile([128, MC, T], FP16)  # y_cat_T, c_in = m*128 + p

            for l in range(HL):
                nc.vector.memset(v_sb[:, :, l * 65 + 64 : l * 65 + 65], 1.0)

            # Preload the exp activation table before any real work
            warm = smpool.tile([1, 2], FP32, tag="warm")
            nc.vector.memset(warm[:], 0.0)
            warm2 = smpool.tile([1, 2], FP16, tag="warm2")
            nc.scalar.activation(
                out=warm2[:], in_=warm[:],
                func=mybir.ActivationFunctionType.Exp, scale=1.0,
            )

            # ---- qkv / proj closures (PE fillers during attention) ----
            def qk_closure(tch, m, w_sb, b_sb, out_sb):
                def emit():
                    tsl = slice(tch * 512, (tch + 1) * 512)
                    msl = slice(m * 128, (m + 1) * 128)
                    pq = miscp.tile([128, 512], FP32, tag="misc")
                    for kc in range(KC):
                        nc.tensor.matmul(
                            pq[:],
                            w_sb[:, kc, msl],
                            xt_sb[:, kc, tsl],
                            start=(kc == 0),
                            stop=(kc == KC - 1),
                        )
                    nc.vector.tensor_scalar_add(
                        out=out_sb[:, m, tsl], in0=pq[:], scalar1=b_sb[:, m : m + 1]
                    )

                return emit

            def v_closure(tch, ts):
                def emit():
                    t16 = tch * 4 + ts
                    pv = miscp.tile([128, 512], FP32, tag="misc")
                    for kc in range(KC):
                        nc.tensor.matmul(
                            pv[:],
                            xt_sb[:, kc, t16 * 128 : (t16 + 1) * 128],
                            wv_sb[:, kc, :],
                            start=(kc == 0),
                            stop=(kc == KC - 1),
                        )
                    # scatter into 65-wide slots (even/odd strided copies) + bias
                    for par in range(2):
                        src = bass.AP(
                            tensor=pv.tensor,
                            offset=pv.offset + par * 64,
                            ap=[pv.ap[0], [128, 4], [1, 64]],
                        )
                        srcb = bass.AP(
                            tensor=bv_sb.tensor,
                            offset=bv_sb.offset + par * 64,
                            ap=[bv_sb.ap[0], [128, 4], [1, 64]],
                        )
                        base = v_sb[:, t16]
                        dst = bass.AP(
                            tensor=base.tensor,
                            offset=base.offset + par * 65,
                            ap=[base.ap[0], [130, 4], [1, 64]],
                        )
                        nc.vector.tensor_add(out=dst, in0=src, in1=srcb)

                return emit

            def proj_closure(tch, mo):
                def emit():
                    tsl = slice(tch * 512, (tch + 1) * 512)
                    osl = slice(mo * 128, (mo + 1) * 128)
                    pp = miscp.tile([128, 512], FP32, tag="misc")
                    for kcm in range(MC):
                        nc.tensor.matmul(
                            pp[:],
                            wp_sb[:, kcm, osl],
                            y_sb[:, kcm, tsl],
                            start=(kcm == 0),
                            stop=(kcm == MC - 1),
                        )
                    yo_sb = yopool.tile([128, 512], FP16, tag="yo")
                    nc.vector.tensor_copy(yo_sb[:], pp[:])
                    # stores on the (otherwise idle) gpsimd SWDGE queue: on the
                    # sync HWDGE queue they can overlap in flight with the
                    # xbar transposes and produced wrong results on HW.  The
                    # final chunk's stores go on the scalar HWDGE queue (ACT
                    # is finished by then) so the tail isn't SWDGE-drain-bound.
                    if tch == NT - 1:
                        nc.scalar.dma_start(out=yp[osl, tsl], in_=yo_sb[:])
                    else:
                        nc.gpsimd.dma_start(out=yp[osl, tsl], in_=yo_sb[:])

                return emit

            def warm_pe(n):
                # tiny no-dep matmuls keep the HAM activity window busy so
                # surrounding real matmuls run at 2.4 GHz instead of 1.2
                wps = miscp.tile([1, 16], FP32, tag="misc")
                ones = v_sb[0:64, 0, 64:65]
                for i in range(n):
                    nc.tensor.matmul(
                        wps[0:1, i % 16 : i % 16 + 1],
                        ones,
                        ones,
                        start=True,
                        stop=True,
                        skip_group_check=True,
                    )

            # ---- startup: minimal qkv before attention ----
            warm_pe(170)
            qk_closure(0, 0, wq_sb, bq_sb, qT_sb)()
            qk_closure(0, 0, wk_sb, bk_sb, kT_sb)()
            for ts in (3, 2):
                v_closure(0, ts)()

            # ---- filler schedule ----
            filler_lists = {
                0: (
                    [v_closure(0, 1), v_closure(0, 0)]
                    + [qk_closure(0, m, wq_sb, bq_sb, qT_sb) for m in (1,)]
                    + [qk_closure(0, m, wk_sb, bk_sb, kT_sb) for m in (1,)]
                    + [qk_closure(0, m, wq_sb, bq_sb, qT_sb) for m in (2,)]
                    + [qk_closure(0, m, wk_sb, bk_sb, kT_sb) for m in (2,)]
                    + [qk_closure(0, m, wq_sb, bq_sb, qT_sb) for m in (3,)]
                    + [qk_closure(0, m, wk_sb, bk_sb, kT_sb) for m in (3,)]
                    + [v_closure(1, ts) for ts in (3, 2, 1, 0)]
                    + [
                        cl
                        for m in range(MC)
                        for cl in (
                            qk_closure(1, m, wq_sb, bq_sb, qT_sb),
                            qk_closure(1, m, wk_sb, bk_sb, kT_sb),
                        )
                    ]
                ),
                1: (
                    [
                        cl
                        for m in range(2)
                        for cl in (
                            qk_closure(2, m, wq_sb, bq_sb, qT_sb),
                            qk_closure(2, m, wk_sb, bk_sb, kT_sb),
                        )
                    ]
                    + [v_closure(2, ts) for ts in (3, 2)]
                    + [
                        cl
                        for m in range(2, MC)
                        for cl in (
                            qk_closure(2, m, wq_sb, bq_sb, qT_sb),
                            qk_closure(2, m, wk_sb, bk_sb, kT_sb),
                        )
                    ]
                    + [v_closure(2, ts) for ts in (1, 0)]
                ),
                2: (
                    [
                        cl
                        for m in range(2)
                        for cl in (
                            qk_closure(3, m, wq_sb, bq_sb, qT_sb),
                            qk_closure(3, m, wk_sb, bk_sb, kT_sb),
                        )
                    ]
                    + [v_closure(3, ts) for ts in (3, 2)]
                    + [
                        cl
                        for m in range(2, MC)
                        for cl in (
                            qk_closure(3, m, wq_sb, bq_sb, qT_sb),
                            qk_closure(3, m, wk_sb, bk_sb, kT_sb),
                        )
                    ]
                    + [v_closure(3, ts) for ts in (1, 0)]
                ),
                3: (
                    [proj_closure(0, mo) for mo in range(C // 128)]
                    + [proj_closure(1, mo) for mo in range(C // 128)]
                    + [proj_closure(2, mo) for mo in range(C // 128)]
                ),
            }

            # ---- attention emission: descending j, deferred pair tails ----
            def make_tail(pavA, pavB, deferred_avs, pair, c, final=False):
                def emit():
                    for av in deferred_avs:
                        av()
                    if final:
                        warm_pe(40)
                    rec = smpool.tile([128, 8], FP32, tag="rec")
                    recA = bass.AP(
                        tensor=pavA.tensor,
                        offset=pavA.offset + 64,
                        ap=[pavA.ap[0], [65, 4]],
                    )
                    recB = bass.AP(
                        tensor=pavB.tensor,
                        offset=pavB.offset + 64,
                        ap=[pavB.ap[0], [65, 4]],
                    )
                    nc.vector.reciprocal(out=rec[:, 0:4], in_=recA)
                    nc.vector.reciprocal(out=rec[:, 4:8], in_=recB)
                    yn = smpool.tile([128, 4, 128], FP16, tag="yn")
                    for k in range(4):
                        nc.vector.tensor_scalar_mul(
                            out=yn[:, k, 0:64],
                            in0=pavA[:, k * 65 : k * 65 + 64],
                            scalar1=rec[:, k : k + 1],
                        )
                        nc.vector.tensor_scalar_mul(
                            out=yn[:, k, 64:128],
                            in0=pavB[:, k * 65 : k * 65 + 64],
                            scalar1=rec[:, 4 + k : 5 + k],
                        )
                    for k in range(4):
                        ysl = y_sb[:, pair, c * 512 + k * 128 : c * 512 + (k + 1) * 128]
                        if final:
                            # last pair: PE transpose + DVE copy, so the
                            # trailing projection has no DMA-queue deps
                            pt = pavp.tile([128, 128], FP16, tag="pav")
                            nc.tensor.transpose(out=pt[:], in_=yn[:, k], identity=id_sb)
                            nc.vector.tensor_copy(ysl, pt[:])
                        else:
                            # SBUF->SBUF xbar DMA transpose [tq,128] -> [c',tq]
                            nc.sync.dma_start(out=ysl, in_=yn[:, k], transpose=True)

                return emit

            pending_tail = None

            for c in range(NT):
                fillers = deque(filler_lists[c])
                nj = 4 * c + 4
                total_iters = NPAIR * nj
                nfill = len(fillers)
                it = 0
                popped = 0

                for pair in range(NPAIR):
                    pavA = pavp.tile([128, 512], FP32, tag="pav")
                    pavB = pavp.tile([128, 512], FP32, tag="pav")
                    deferred = None
                    for idx, j in enumerate(range(nj - 1, -1, -1)):
                        off = max(0, 128 * j - 512 * c)  # multiple of 128
                        far = j <= 4 * c - 2
                        s = 512 * c - 128 * j + 384

                        # PE fillers BEFORE the S pair: when S is ACT-paced,
                        # the fillers run during its wait instead of queuing
                        # behind it.  In chunk 3 the fillers are proj closures
                        # whose DMA-queue semaphore targets count the freshly
                        # issued pair-tail transposes (false dependency); hold
                        # pops until idx>=4 so those have drained.
                        # (chunk 3 fillers drain 8 iters early so their CASTs
                        # clear the DVE queue before the final pair's tail;
                        # pops are gated off idx<=4 — the pair-tail transposes
                        # issue late behind the DVE and take ~7us to drain —
                        # and off the chunk entry so the first S isn't delayed)
                        it += 1
                        lead = 8 if c == 3 else 0
                        due = nfill * min(it + lead, total_iters) // total_iters
                        gated = c == 3 and (1 <= idx <= 4 or (pair == 0 and idx == 0))
                        if not gated:
                            while popped < due:
                                fillers.popleft()()
                                popped += 1

                        pS = ps.tile([128, 1024], FP32, tag="pS")
                        nc.tensor.matmul(
                            pS[:, off:512],
                            kT_sb[0:64, pair, j * 128 : (j + 1) * 128],
                            qT_sb[0:64, pair, c * 512 + off : (c + 1) * 512],
                            start=True,
                            stop=True,
                        )
                        nc.tensor.matmul(
                            pS[:, 512 + off : 1024],
                            kT_sb[64:128, pair, j * 128 : (j + 1) * 128],
                            qT_sb[64:128, pair, c * 512 + off : (c + 1) * 512],
                            start=True,
                            stop=True,
                        )

                        # previous pair's deferred tail: after this pair's
                        # first S so the PE queue never head-blocks on exp
                        if idx == 0 and pending_tail is not None:
                            pending_tail()
                            pending_tail = None

                        p2 = p2pool.tile([128, 1024], FP16, tag="p2")
                        # 2-level AP skips head-B's dead zone [512:512+off]
                        # (never written by S; saves ACT cycles on diag tiles)
                        exp_in = bass.AP(
                            tensor=pS.tensor,
                            offset=pS.offset + off,
                            ap=[pS.ap[0], [512, 2], [1, 512 - off]],
                        )
                        exp_out = bass.AP(
                            tensor=p2.tensor,
                            offset=p2.offset + off,
                            ap=[p2.ap[0], [512, 2], [1, 512 - off]],
                        )
                        nc.scalar.activation(
                            out=exp_out,
                            in_=exp_in,
                            func=mybir.ActivationFunctionType.Exp,
                            scale=1.0 / math.sqrt(D),
                        )
                        if far:
                            pmt = p2
                        else:
                            pmt = pmpool.tile([128, 1024], FP16, tag="pm")
                            ea_pair = bass.AP(
                                tensor=ea_sb[pair].tensor,
                                offset=ea_sb[pair].offset + s + off,
                                ap=[ea_sb[pair].ap[0], [EA_W, 2], [1, 512 - off]],
                            )
                            dst = bass.AP(
                                tensor=pmt.tensor,
                                offset=pmt.offset + off,
                                ap=[pmt.ap[0], [512, 2], [1, 512 - off]],
                            )
                            srcp = bass.AP(
                                tensor=p2.tensor,
                                offset=p2.offset + off,
                                ap=[p2.ap[0], [512, 2], [1, 512 - off]],
                            )
                            # (masks stay on DVE: GPSIMD tensor_mul measured
                            # ~2x slower and serializes behind the yp stores)
                            nc.vector.tensor_mul(out=dst, in0=srcp, in1=ea_pair)

                        def av_emit(
                            pmt=pmt, j=j, off=off, idx=idx, pavA=pavA, pavB=pavB, pair=pair
                        ):
                            # start=True clears the has_written map of the
                            # WHOLE PSUM bank; with descending j the first
                            # emitted AV (j=nj-1, k=3) sets it and every later
                            # matmul overwrites-where-clear / accumulates.
                            # stop closes each subtile's group at its true
                            # last contribution (j == 0; sim-only).
                            for k in range(off // 128, 4):
                                nc.tensor.matmul(
                                    pavA[:, k * 65 : k * 65 + 65],
                                    pmt[:, k * 128 : (k + 1) * 128],
                                    v_sb[:, j, 2 * pair * 65 : 2 * pair * 65 + 65],
                                    start=(idx == 0),
                                    stop=(j == 0),
                                    skip_group_check=True,
                                )
                                nc.tensor.matmul(
                                    pavB[:, k * 65 : k * 65 + 65],
                                    pmt[:, 512 + k * 128 : 512 + (k + 1) * 128],
                                    v_sb[
                                        :, j, (2 * pair + 1) * 65 : (2 * pair + 1) * 65 + 65
                                    ],
                                    start=(idx == 0),
                                    stop=(j == 0),
                                    skip_group_check=True,
                                )

                        # the very last pair has no successor to pipeline
                        # into, so emit its final AV inline
                        if idx < nj - 1 or (c == NT - 1 and pair == NPAIR - 1):
                            av_emit()
                            deferred = deferred or []
                        else:
                            deferred = [av_emit]

                    pending_tail = make_tail(
                        pavA, pavB, deferred, pair, c,
                        final=(c == NT - 1 and pair == NPAIR - 1),
                    )

                while fillers:
                    fillers.popleft()()

            # final pair tail + last chunk's projection (kcm 0-2 of each mo
            # overlap the tail transposes; only kcm=3 waits on them)
            pending_tail()
            for mo in range(C // 128):
                proj_closure(NT - 1, mo)()

    nc.compile()
    return nc


_NC = None
LAST_RESULTS = None


def _get_program():
    global _NC
    if _NC is None:
        _NC = _build_program()
    return _NC


# Bucket b covers distances d in [starts[b], starts[b+1]); verified bit-exact
# against the jax reference's _relative_position_bucket for T=2048.
_BUCKET_STARTS = np.array(
    [0, 1, 2, 3, 4, 5, 6, 7, 8, 9, 10, 11, 12, 13, 14, 15,
     16, 18, 20, 23, 26, 29, 33, 38, 43, 49, 55, 63, 72, 82, 93, 106]
)


def _rel_bias_buckets():
    """bucket(d) for d = q - k in [0, T)."""
    d = np.arange(T)
    return np.searchsorted(_BUCKET_STARTS, d, side="right") - 1


def _make_in_maps(x, W_attn, b_attn, W_proj, rel_emb):
    buckets = _rel_bias_buckets()  # [T]
    bias_by_dist = rel_emb[buckets, :]  # [T, H] fp32
    # Divide by exp(b31) per head: far tiles (all d >= 106, bucket 31) then
    # multiply by exactly 1.0 and can skip the mask-multiply; the factor
    # cancels in the softmax ratio.
    b31 = rel_emb[NUM_BUCKETS - 1, :]  # [H]
    # vec[h, j] = exp(bias[j - 511] - b31[h]) for j >= 511 else 0.  Only
    # distances d < EA_W - 384 ever appear in near tiles (d = s + tq - p - 384
    # with s <= 512), so the table is clipped to EA_W + 127 columns.
    ndist = EA_W + 127 - 511  # 640
    vec = np.zeros((H, EA_W + 127), dtype=np.float32)
    vec[:, 511:] = np.exp(bias_by_dist[:ndist].T - b31[:, None])
    vec = vec.astype(np.float16)
    # expand to the per-head Toeplitz table A[h, p, x] = vec[h, x - p + 127]
    sw = np.lib.stride_tricks.sliding_window_view(vec, EA_W, axis=1)  # [H,128,EA_W]
    wexp_all = np.ascontiguousarray(sw[:, ::-1, :])  # [H, 128, EA_W]

    def pmajor(a, nblk):
        """[nblk*128, M] -> [128, nblk, M] partition-major contiguous fp16."""
        a = np.asarray(a)
        return np.ascontiguousarray(
            a.reshape(nblk, 128, a.shape[1]).transpose(1, 0, 2)
        ).astype(np.float16)

    def pm_mmajor(a):
        """[KC*128, MC*128] -> [128, MC, KC, 128] (m-major slices)."""
        a = np.asarray(a)
        return np.ascontiguousarray(
            a.reshape(KC, 128, MC, 128).transpose(1, 2, 0, 3)
        ).astype(np.float16)

    in_maps = []
    for core in range(NCORES):
        b, hg = core // 2, core % 2
        csl = slice(hg * CL, (hg + 1) * CL)
        in_maps.append(
            {
                "xh": pmajor(x[b].T.astype(np.float16), KC),
                "wq": pm_mmajor(W_attn[csl, :].T.astype(np.float16)),
                "wk": pm_mmajor(
                    W_attn[C + hg * CL : C + (hg + 1) * CL, :].T.astype(np.float16)
                ),
                "wv": pmajor(
                    W_attn[2 * C + hg * CL : 2 * C + (hg + 1) * CL, :].T.astype(
                        np.float16
                    ),
                    KC,
                ),
                "wp": pmajor(W_proj[:, csl].T.astype(np.float16), MC),
                "bqk": np.stack(
                    [b_attn[csl], b_attn[C + hg * CL : C + (hg + 1) * CL]]
                ).astype(np.float32),
                "bvr": np.ascontiguousarray(
                    np.broadcast_to(
                        b_attn[2 * C + hg * CL : 2 * C + (hg + 1) * CL].astype(
                            np.float32
                        ),
                        (128, CL),
                    )
                ),
                "wexp": np.ascontiguousarray(
                    wexp_all[hg * HL : (hg + 1) * HL].reshape(NPAIR, 2, 128, EA_W)
                ),
                "ident": np.eye(128, dtype=np.float16),
            }
        )
    return in_maps


def kernel(x, W_attn, b_attn, W_proj, b_proj, rel_emb):
    x = np.asarray(x)
    W_attn = np.asarray(W_attn)
    b_attn = np.asarray(b_attn)
    W_proj = np.asarray(W_proj)
    b_proj = np.asarray(b_proj)
    rel_emb = np.asarray(rel_emb)

    in_maps = _make_in_maps(x, W_attn, b_attn, W_proj, rel_emb)
    nc = _get_program()
    res = bass_utils.run_bass_kernel_spmd(nc, in_maps, core_ids=list(range(NCORES)))
    global LAST_RESULTS
    LAST_RESULTS = res

    y = np.empty((B, T, C), dtype=np.float32)
    for b in range(B):
        ypT = res.results[2 * b]["yp"].astype(np.float32) + res.results[2 * b + 1][
            "yp"
        ].astype(np.float32)
        y[b] = ypT.T + b_proj[None, :].astype(np.float32)
    return y


# revision 48
# speedup vs baseline: 1.0160x; 1.0160x over previous
"""Trainium2 Bass kernel for causal self-attention with T5 relative position bias.

Problem (hardcoded): B=4, T=2048, C=1024, H=16, D=64, NUM_BUCKETS=32, MAX_DISTANCE=128.
Sharding over 8 cores: core c -> (batch b=c//2, head-group hg=c%2 of 8 heads).
Each core computes qkv projection for its heads, causal attention, and a partial
output projection (its heads' rows of W_proj); host sums the two partials per batch.

Key structure:
  - Heads are processed in PAIRS (2m on partitions 0-63, 2m+1 on 64-127); the two
    K=64 S-matmuls of a pair row-tile the PE at (0,0)/(64,0).
  - AV is "flipped": P tiles [tk,128tq] stationary, v+ones [tk,65] moving ->
    out [tq,65]; softmax rowsum lands as a per-partition column so normalization
    is a DVE reciprocal + tensor_scalar multiply.
  - The T5 bias table is host-divided by exp(b31); far tiles skip the mask-mult;
    exp uses a 2-level AP that skips head-B's dead zone on diagonal tiles.
  - Pipeline shape (measured-bottleneck driven; ~258us vs 287us baseline):
    * Inputs are host-pre-arranged partition-major so DMA descriptors are
      >=2KB; loads are priority-ordered across the sync+scalar HW-DGE queues
      (m0 weight slices + x first-quarters first) so the first S matmul lands
      ~28us in; warm-up dummy matmuls keep the PE HAM clock at 2.4 GHz.
    * Within a pair, j (k-tile index) runs DESCENDING (diagonal first); the
      last j's AV + normalize + transposes are DEFERRED into the next pair's
      stream, right after its first S pair, so the PE queue never head-blocks
      on exp at pair boundaries and the ACT engine stays saturated.
    * Fillers run BEFORE each S pair: chunk0: rest of qkv0 + qkv1; chunk1:
      qkv2; chunk2: qkv3; chunk3: proj0-2 (the ACT-bound chunk absorbs proj).
      In chunk 3 pops are gated off the first iterations of each pair to dodge
      a false DMA-counter dependency on the freshly issued pair-tail
      transposes (the HWDGE completion semaphore counts in queue order).
    * yp stores go on the gpsimd SWDGE queue (sharing the sync queue with the
      xbar transposes produced wrong results on HW); the final chunk's stores
      use the scalar queue, free once the last exp is done.
    * Tail: the last pair's y is transposed on the PE (identity matmul), so
      proj(chunk3) has no DMA dependencies and overlaps the tail drain.
"""

import sys

sys.path.insert(0, "/opt/trn_rl_repo")

import math
from collections import deque

import numpy as np

import concourse.bacc as bacc
import concourse.bass as bass
import concourse.mybir as mybir
import concourse.tile as tile
from concourse import bass_utils


def _ensure_axon_hooks():
    """bass_utils imports antenv.axon_hooks when BASS_TRACE is set under axon;
    this image's antenv lacks that submodule. Provide an inert one so a stray
    trace env var degrades to a warning instead of crashing the run."""
    try:
        import antenv.axon_hooks  # noqa: F401
    except Exception:
        try:
            import types

            import antenv

            hooks = types.ModuleType("antenv.axon_hooks")
            hooks._hook = None
            hooks.set_axon_ntff_profile_hook = lambda h: setattr(hooks, "_hook", h)
            hooks.get_axon_ntff_profile_hook = lambda: hooks._hook
            sys.modules["antenv.axon_hooks"] = hooks
            antenv.axon_hooks = hooks
        except Exception:
            pass


_ensure_axon_hooks()

B, T, C = 4, 2048, 1024
H, D = 16, 64
NUM_BUCKETS, MAX_DISTANCE = 32, 128
HL = 8  # local heads per core
CL = HL * D  # 512 local channels
NCORES = 8
NPAIR = HL // 2  # 4 head pairs per core

FP16 = mybir.dt.float16
FP32 = mybir.dt.float32

NT = T // 512  # 4 tq chunks of 512
NK = T // 128  # 16 tk tiles of 128
KC = C // 128  # 8 contraction chunks for qkv
MC = CL // 128  # 4 m-chunks of local channels

# ea table geometry: slice start s = (tq0 - tk0) + 384; near tiles only, so
# s in {0,128,256,384,512} and max used column is s+511 = 1023.
EA_W = 1024


def _build_program():
    nc = bacc.Bacc(None, target_bir_lowering=False)

    # Inputs are host-pre-arranged partition-major ([128, kc, ...]) so every
    # DMA descriptor covers >=2KB contiguous per partition row.  wq/wk are
    # additionally m-major so the m=0 slices (the only ones the startup
    # q/k chains need) can be prioritized on the load queues.
    xh = nc.dram_tensor("xh", [128, KC, T], FP16, kind="ExternalInput")
    wq = nc.dram_tensor("wq", [128, MC, KC, 128], FP16, kind="ExternalInput")
    wk = nc.dram_tensor("wk", [128, MC, KC, 128], FP16, kind="ExternalInput")
    wv = nc.dram_tensor("wv", [128, KC, CL], FP16, kind="ExternalInput")
    wp = nc.dram_tensor("wp", [128, MC, C], FP16, kind="ExternalInput")
    bqk = nc.dram_tensor("bqk", [2, CL], FP32, kind="ExternalInput")
    bvr = nc.dram_tensor("bvr", [128, CL], FP32, kind="ExternalInput")
    # per-PAIR tables: [pair, head-in-pair, 128, EA_W], host-divided by exp(b31)
    wexp = nc.dram_tensor("wexp", [NPAIR, 2, 128, EA_W], FP16, kind="ExternalInput")
    ident = nc.dram_tensor("ident", [128, 128], FP16, kind="ExternalInput")
    yp = nc.dram_tensor("yp", [C, T], FP16, kind="ExternalOutput")

    with tile.TileContext(nc) as tc:
        with (
            tc.tile_pool(name="w", bufs=1) as wpool,
            tc.tile_pool(name="big", bufs=1) as bigpool,
            tc.tile_pool(name="ea", bufs=1) as eapool,
            tc.tile_pool(name="p2", bufs=6) as p2pool,
            tc.tile_pool(name="pm", bufs=6) as pmpool,
            tc.tile_pool(name="sm", bufs=2) as smpool,
            tc.tile_pool(name="yo", bufs=6) as yopool,
            tc.tile_pool(name="ps", bufs=2, space="PSUM") as ps,
            tc.tile_pool(name="pav", bufs=2, space="PSUM") as pavp,
            tc.tile_pool(name="misc", bufs=2, space="PSUM") as miscp,
        ):
            # ---- weights / constants ----
            wq_sb = wpool.tile([128, KC, CL], FP16)
            wk_sb = wpool.tile([128, KC, CL], FP16)
            wv_sb = wpool.tile([128, KC, CL], FP16)
            wp_sb = wpool.tile([128, MC, C], FP16)
            bq_sb = wpool.tile([128, MC], FP32)
            bk_sb = wpool.tile([128, MC], FP32)
            bv_sb = wpool.tile([128, CL], FP32)
            xt_sb = bigpool.tile([128, KC, T], FP16)
            ea_sb = [
                eapool.tile([128, 2, EA_W], FP16, name=f"ea{p}") for p in range(NPAIR)
            ]

            bqk_r = bqk.rearrange("b (m p) -> b p m", p=128)
            wexp_r = wexp.rearrange("pr h p w -> pr p h w")

            # Priority-ordered loads split over the two HW-DGE queues
            # (sync + scalar), most-critical bytes first: the m=0 q/k weight
            # slices, then x (split across both queues), then wv/ea0 (first
            # pair's AV + mask), then the remaining weight slices.
            id_sb = wpool.tile([128, 128], FP16)

            nc.sync.dma_start(out=bq_sb, in_=bqk_r[0])
            nc.sync.dma_start(out=bk_sb, in_=bqk_r[1])
            nc.sync.dma_start(out=wq_sb[:, :, 0:128], in_=wq[:, 0])
            for kc in range(4):
                nc.sync.dma_start(out=xt_sb[:, kc, 0:512], in_=xh[:, kc, 0:512])
            nc.sync.dma_start(out=wv_sb, in_=wv[:])
            nc.sync.dma_start(out=bv_sb, in_=bvr[:])
            for m in range(1, MC):
                nc.sync.dma_start(
                    out=wq_sb[:, :, m * 128 : (m + 1) * 128], in_=wq[:, m]
                )
            for kc in range(4):
                nc.sync.dma_start(out=xt_sb[:, kc, 512:1024], in_=xh[:, kc, 512:1024])
            for kc in range(KC):
                nc.sync.dma_start(
                    out=xt_sb[:, kc, 1024:2048], in_=xh[:, kc, 1024:2048]
                )
            nc.sync.dma_start(out=wp_sb, in_=wp[:])
            nc.sync.dma_start(out=id_sb, in_=ident[:])

            nc.scalar.dma_start(out=wk_sb[:, :, 0:128], in_=wk[:, 0])
            for kc in range(4, KC):
                nc.scalar.dma_start(out=xt_sb[:, kc, 0:512], in_=xh[:, kc, 0:512])
            for kc in range(4, KC):
                nc.scalar.dma_start(out=xt_sb[:, kc, 512:1024], in_=xh[:, kc, 512:1024])
            nc.scalar.dma_start(out=ea_sb[0], in_=wexp_r[0])
            for m in range(1, MC):
                nc.scalar.dma_start(
                    out=wk_sb[:, :, m * 128 : (m + 1) * 128], in_=wk[:, m]
                )
                nc.scalar.dma_start(out=ea_sb[m], in_=wexp_r[m])

            # ---- persistent activations ----
            qT_sb = bigpool.tile([128, MC, T], FP16)  # c' = m*128 + p
            kT_sb = bigpool.tile([128, MC, T], FP16)
            v_sb = bigpool.tile([128, NK, HL * 65], FP16)  # slot l: [v(64), ones]
            y_sb = bigpool.tile([128, MC, T], FP16)  # y_cat_T, c_in = m*128 + p

            for l in range(HL):
                nc.vector.memset(v_sb[:, :, l * 65 + 64 : l * 65 + 65], 1.0)

            # Preload the exp activation table before any real work
            warm = smpool.tile([1, 2], FP32, tag="warm")
            nc.vector.memset(warm[:], 0.0)
            warm2 = smpool.tile([1, 2], FP16, tag="warm2")
            nc.scalar.activation(
                out=warm2[:], in_=warm[:],
                func=mybir.ActivationFunctionType.Exp, scale=1.0,
            )

            # ---- qkv / proj closures (PE fillers during attention) ----
            def qk_closure(tch, m, w_sb, b_sb, out_sb):
                def emit():
                    tsl = slice(tch * 512, (tch + 1) * 512)
                    msl = slice(m * 128, (m + 1) * 128)
                    pq = miscp.tile([128, 512], FP32, tag="misc")
                    for kc in range(KC):
                        nc.tensor.matmul(
                            pq[:],
                            w_sb[:, kc, msl],
                            xt_sb[:, kc, tsl],
                            start=(kc == 0),
                            stop=(kc == KC - 1),
                        )
                    nc.vector.tensor_scalar_add(
                        out=out_sb[:, m, tsl], in0=pq[:], scalar1=b_sb[:, m : m + 1]
                    )

                return emit

            def v_closure(tch, ts):
                def emit():
                    t16 = tch * 4 + ts
                    pv = miscp.tile([128, 512], FP32, tag="misc")
                    for kc in range(KC):
                        nc.tensor.matmul(
                            pv[:],
                            xt_sb[:, kc, t16 * 128 : (t16 + 1) * 128],
                            wv_sb[:, kc, :],
                            start=(kc == 0),
                            stop=(kc == KC - 1),
                        )
                    # scatter into 65-wide slots (even/odd strided copies) + bias
                    for par in range(2):
                        src = bass.AP(
                            tensor=pv.tensor,
                            offset=pv.offset + par * 64,
                            ap=[pv.ap[0], [128, 4], [1, 64]],
                        )
                        srcb = bass.AP(
                            tensor=bv_sb.tensor,
                            offset=bv_sb.offset + par * 64,
                            ap=[bv_sb.ap[0], [128, 4], [1, 64]],
                        )
                        base = v_sb[:, t16]
                        dst = bass.AP(
                            tensor=base.tensor,
                            offset=base.offset + par * 65,
                            ap=[base.ap[0], [130, 4], [1, 64]],
                        )
                        nc.vector.tensor_add(out=dst, in0=src, in1=srcb)

                return emit

            def proj_closure(tch, mo):
                def emit():
                    tsl = slice(tch * 512, (tch + 1) * 512)
                    osl = slice(mo * 128, (mo + 1) * 128)
                    pp = miscp.tile([128, 512], FP32, tag="misc")
                    for kcm in range(MC):
                        nc.tensor.matmul(
                            pp[:],
                            wp_sb[:, kcm, osl],
                            y_sb[:, kcm, tsl],
                            start=(kcm == 0),
                            stop=(kcm == MC - 1),
                        )
                    yo_sb = yopool.tile([128, 512], FP16, tag="yo")
                    nc.vector.tensor_copy(yo_sb[:], pp[:])
                    # stores on the (otherwise idle) gpsimd SWDGE queue: on the
                    # sync HWDGE queue they can overlap in flight with the
                    # xbar transposes and produced wrong results on HW.  The
                    # final chunk's stores go on the scalar HWDGE queue (ACT
                    # is finished by then) so the tail isn't SWDGE-drain-bound.
                    if tch == NT - 1:
                        nc.scalar.dma_start(out=yp[osl, tsl], in_=yo_sb[:])
                    else:
                        nc.gpsimd.dma_start(out=yp[osl, tsl], in_=yo_sb[:])

                return emit

            def warm_pe(n):
                # tiny no-dep matmuls keep the HAM activity window busy so
                # surrounding real matmuls run at 2.4 GHz instead of 1.2
                wps = miscp.tile([1, 16], FP32, tag="misc")
                ones = v_sb[0:64, 0, 64:65]
                for i in range(n):
                    nc.tensor.matmul(
                        wps[0:1, i % 16 : i % 16 + 1],
                        ones,
                        ones,
                        start=True,
                        stop=True,
                        skip_group_check=True,
                    )

            # ---- startup: minimal qkv before attention ----
            warm_pe(170)
            qk_closure(0, 0, wq_sb, bq_sb, qT_sb)()
            qk_closure(0, 0, wk_sb, bk_sb, kT_sb)()
            for ts in (3, 2):
                v_closure(0, ts)()

            # ---- filler schedule ----
            filler_lists = {
                0: (
                    [v_closure(0, 1), v_closure(0, 0)]
                    + [qk_closure(0, m, wq_sb, bq_sb, qT_sb) for m in (1,)]
                    + [qk_closure(0, m, wk_sb, bk_sb, kT_sb) for m in (1,)]
                    + [qk_closure(0, m, wq_sb, bq_sb, qT_sb) for m in (2,)]
                    + [qk_closure(0, m, wk_sb, bk_sb, kT_sb) for m in (2,)]
                    + [qk_closure(0, m, wq_sb, bq_sb, qT_sb) for m in (3,)]
                    + [qk_closure(0, m, wk_sb, bk_sb, kT_sb) for m in (3,)]
                    + [v_closure(1, ts) for ts in (3, 2, 1, 0)]
                    + [
                        cl
                        for m in range(MC)
                        for cl in (
                            qk_closure(1, m, wq_sb, bq_sb, qT_sb),
                            qk_closure(1, m, wk_sb, bk_sb, kT_sb),
                        )
                    ]
                ),
                1: (
                    [
                        cl
                        for m in range(2)
                        for cl in (
                            qk_closure(2, m, wq_sb, bq_sb, qT_sb),
                            qk_closure(2, m, wk_sb, bk_sb, kT_sb),
                        )
                    ]
                    + [v_closure(2, ts) for ts in (3, 2)]
                    + [
                        cl
                        for m in range(2, MC)
                        for cl in (
                            qk_closure(2, m, wq_sb, bq_sb, qT_sb),
                            qk_closure(2, m, wk_sb, bk_sb, kT_sb),
                        )
                    ]
                    + [v_closure(2, ts) for ts in (1, 0)]
                ),
                2: (
                    [
                        cl
                        for m in range(2)
                        for cl in (
                            qk_closure(3, m, wq_sb, bq_sb, qT_sb),
                            qk_closure(3, m, wk_sb, bk_sb, kT_sb),
                        )
                    ]
                    + [v_closure(3, ts) for ts in (3, 2)]
                    + [
                        cl
                        for m in range(2, MC)
                        for cl in (
                            qk_closure(3, m, wq_sb, bq_sb, qT_sb),
                            qk_closure(3, m, wk_sb, bk_sb, kT_sb),
                        )
                    ]
                    + [v_closure(3, ts) for ts in (1, 0)]
                ),
                3: (
                    [proj_closure(0, mo) for mo in range(C // 128)]
                    + [proj_closure(1, mo) for mo in range(C // 128)]
                    + [proj_closure(2, mo) for mo in range(C // 128)]
                ),
            }

            # ---- attention emission: descending j, deferred pair tails ----
            def make_tail(pavA, pavB, deferred_avs, pair, c, final=False):
                def emit():
                    for av in deferred_avs:
                        av()
                    if final:
                        warm_pe(40)
                    rec = smpool.tile([128, 8], FP32, tag="rec")
                    recA = bass.AP(
                        tensor=pavA.tensor,
                        offset=pavA.offset + 64,
                        ap=[pavA.ap[0], [65, 4]],
                    )
                    recB = bass.AP(
                        tensor=pavB.tensor,
                        offset=pavB.offset + 64,
                        ap=[pavB.ap[0], [65, 4]],
                    )
                    nc.vector.reciprocal(out=rec[:, 0:4], in_=recA)
                    nc.vector.reciprocal(out=rec[:, 4:8], in_=recB)
                    yn = smpool.tile([128, 4, 128], FP16, tag="yn")
                    for k in range(4):
                        nc.vector.tensor_scalar_mul(
                            out=yn[:, k, 0:64],
                            in0=pavA[:, k * 65 : k * 65 + 64],
                            scalar1=rec[:, k : k + 1],
                        )
                        nc.vector.tensor_scalar_mul(
                            out=yn[:, k, 64:128],
                            in0=pavB[:, k * 65 : k * 65 + 64],
                            scalar1=rec[:, 4 + k : 5 + k],
                        )
                    for k in range(4):
                        ysl = y_sb[:, pair, c * 512 + k * 128 : c * 512 + (k + 1) * 128]
                        if final:
                            # last pair: PE transpose + DVE copy, so the
                            # trailing projection has no DMA-queue deps
                            pt = pavp.tile([128, 128], FP16, tag="pav")
                            nc.tensor.transpose(out=pt[:], in_=yn[:, k], identity=id_sb)
                            nc.vector.tensor_copy(ysl, pt[:])
                        else:
                            # SBUF->SBUF xbar DMA transpose [tq,128] -> [c',tq]
                            nc.sync.dma_start(out=ysl, in_=yn[:, k], transpose=True)

                return emit

            pending_tail = None

            for c in range(NT):
                fillers = deque(filler_lists[c])
                nj = 4 * c + 4
                total_iters = NPAIR * nj
                nfill = len(fillers)
                it = 0
                popped = 0

                for pair in range(NPAIR):
                    pavA = pavp.tile([128, 512], FP32, tag="pav")
                    pavB = pavp.tile([128, 512], FP32, tag="pav")
                    deferred = None
                    for idx, j in enumerate(range(nj - 1, -1, -1)):
                        off = max(0, 128 * j - 512 * c)  # multiple of 128
                        far = j <= 4 * c - 2
                        s = 512 * c - 128 * j + 384

                        # PE fillers BEFORE the S pair: when S is ACT-paced,
                        # the fillers run during its wait instead of queuing
                        # behind it.  In chunk 3 the fillers are proj closures
                        # whose DMA-queue semaphore targets count the freshly
                        # issued pair-tail transposes (false dependency); hold
                        # pops until idx>=4 so those have drained.
                        # (chunk 3 fillers drain 8 iters early so their CASTs
                        # clear the DVE queue before the final pair's tail;
                        # pops are gated off idx<=4 — the pair-tail transposes
                        # issue late behind the DVE and take ~7us to drain —
                        # and off the chunk entry so the first S isn't delayed)
                        it += 1
                        lead = 8 if c == 3 else 0
                        due = nfill * min(it + lead, total_iters) // total_iters
                        gated = c == 3 and (1 <= idx <= 4 or (pair == 0 and idx == 0))
                        if not gated:
                            while popped < due:
                                fillers.popleft()()
                                popped += 1

                        pS = ps.tile([128, 1024], FP32, tag="pS")
                        nc.tensor.matmul(
                            pS[:, off:512],
                            kT_sb[0:64, pair, j * 128 : (j + 1) * 128],
                            qT_sb[0:64, pair, c * 512 + off : (c + 1) * 512],
                            start=True,
                            stop=True,
                        )
                        nc.tensor.matmul(
                            pS[:, 512 + off : 1024],
                            kT_sb[64:128, pair, j * 128 : (j + 1) * 128],
                            qT_sb[64:128, pair, c * 512 + off : (c + 1) * 512],
                            start=True,
                            stop=True,
                        )

                        # previous pair's deferred tail: after this pair's
                        # first S so the PE queue never head-blocks on exp
                        if idx == 0 and pending_tail is not None:
                            pending_tail()
                            pending_tail = None

                        p2 = p2pool.tile([128, 1024], FP16, tag="p2")
                        # 2-level AP skips head-B's dead zone [512:512+off]
                        # (never written by S; saves ACT cycles on diag tiles)
                        exp_in = bass.AP(
                            tensor=pS.tensor,
                            offset=pS.offset + off,
                            ap=[pS.ap[0], [512, 2], [1, 512 - off]],
                        )
                        exp_out = bass.AP(
                            tensor=p2.tensor,
                            offset=p2.offset + off,
                            ap=[p2.ap[0], [512, 2], [1, 512 - off]],
                        )
                        nc.scalar.activation(
                            out=exp_out,
                            in_=exp_in,
                            func=mybir.ActivationFunctionType.Exp,
                            scale=1.0 / math.sqrt(D),
                        )
                        if far:
                            pmt = p2
                        else:
                            pmt = pmpool.tile([128, 1024], FP16, tag="pm")
                            ea_pair = bass.AP(
                                tensor=ea_sb[pair].tensor,
                                offset=ea_sb[pair].offset + s + off,
                                ap=[ea_sb[pair].ap[0], [EA_W, 2], [1, 512 - off]],
                            )
                            dst = bass.AP(
                                tensor=pmt.tensor,
                                offset=pmt.offset + off,
                                ap=[pmt.ap[0], [512, 2], [1, 512 - off]],
                            )
                            srcp = bass.AP(
                                tensor=p2.tensor,
                                offset=p2.offset + off,
                                ap=[p2.ap[0], [512, 2], [1, 512 - off]],
                            )
                            # (masks stay on DVE: GPSIMD tensor_mul measured
                            # ~2x slower and serializes behind the yp stores)
                            nc.vector.tensor_mul(out=dst, in0=srcp, in1=ea_pair)

                        def av_emit(
                            pmt=pmt, j=j, off=off, idx=idx, pavA=pavA, pavB=pavB, pair=pair
                        ):
                            # start=True clears the has_written map of the
                            # WHOLE PSUM bank; with descending j the first
                            # emitted AV (j=nj-1, k=3) sets it and every later
                            # matmul overwrites-where-clear / accumulates.
                            # stop closes each subtile's group at its true
                            # last contribution (j == 0; sim-only).
                            for k in range(off // 128, 4):
                                nc.tensor.matmul(
                                    pavA[:, k * 65 : k * 65 + 65],
                                    pmt[:, k * 128 : (k + 1) * 128],
                                    v_sb[:, j, 2 * pair * 65 : 2 * pair * 65 + 65],
                                    start=(idx == 0),
                                    stop=(j == 0),
                                    skip_group_check=True,
                                )
                                nc.tensor.matmul(
                                    pavB[:, k * 65 : k * 65 + 65],
                                    pmt[:, 512 + k * 128 : 512 + (k + 1) * 128],
                                    v_sb[
                                        :, j, (2 * pair + 1) * 65 : (2 * pair + 1) * 65 + 65
                                    ],
                                    start=(idx == 0),
                                    stop=(j == 0),
                                    skip_group_check=True,
                                )

                        # the very last pair has no successor to pipeline
                        # into, so emit its final AV inline
                        if idx < nj - 1 or (c == NT - 1 and pair == NPAIR - 1):
                            av_emit()
                            deferred = deferred or []
                        else:
                            deferred = [av_emit]

                    pending_tail = make_tail(
                        pavA, pavB, deferred, pair, c,
                        final=(c == NT - 1 and pair == NPAIR - 1),
                    )

                while fillers:
                    fillers.popleft()()

            # final pair tail + last chunk's projection (kcm 0-2 of each mo
            # overlap the tail transposes; only kcm=3 waits on them)
            pending_tail()
            for mo in range(C // 128):
                proj_closure(NT - 1, mo)()

    nc.compile()
    return nc


_NC = None
LAST_RESULTS = None


def _get_program():
    global _NC
    if _NC is None:
        _NC = _build_program()
    return _NC


# Bucket b covers distances d in [starts[b], starts[b+1]); verified bit-exact
# against the jax reference's _relative_position_bucket for T=2048.
_BUCKET_STARTS = np.array(
    [0, 1, 2, 3, 4, 5, 6, 7, 8, 9, 10, 11, 12, 13, 14, 15,
     16, 18, 20, 23, 26, 29, 33, 38, 43, 49, 55, 63, 72, 82, 93, 106]
)


def _rel_bias_buckets():
    """bucket(d) for d = q - k in [0, T)."""
    d = np.arange(T)
    return np.searchsorted(_BUCKET_STARTS, d, side="right") - 1


def _make_in_maps(x, W_attn, b_attn, W_proj, rel_emb):
    buckets = _rel_bias_buckets()  # [T]
    bias_by_dist = rel_emb[buckets, :]  # [T, H] fp32
    # Divide by exp(b31) per head: far tiles (all d >= 106, bucket 31) then
    # multiply by exactly 1.0 and can skip the mask-multiply; the factor
    # cancels in the softmax ratio.
    b31 = rel_emb[NUM_BUCKETS - 1, :]  # [H]
    # vec[h, j] = exp(bias[j - 511] - b31[h]) for j >= 511 else 0.  Only
    # distances d < EA_W - 384 ever appear in near tiles (d = s + tq - p - 384
    # with s <= 512), so the table is clipped to EA_W + 127 columns.
    ndist = EA_W + 127 - 511  # 640
    vec = np.zeros((H, EA_W + 127), dtype=np.float32)
    vec[:, 511:] = np.exp(bias_by_dist[:ndist].T - b31[:, None])
    vec = vec.astype(np.float16)
    # expand to the per-head Toeplitz table A[h, p, x] = vec[h, x - p + 127]
    sw = np.lib.stride_tricks.sliding_window_view(vec, EA_W, axis=1)  # [H,128,EA_W]
    wexp_all = np.ascontiguousarray(sw[:, ::-1, :])  # [H, 128, EA_W]

    def pmajor(a, nblk):
        """[nblk*128, M] -> [128, nblk, M] partition-major contiguous fp16."""
        a = np.asarray(a)
        return np.ascontiguousarray(
            a.reshape(nblk, 128, a.shape[1]).transpose(1, 0, 2)
        ).astype(np.float16)

    def pm_mmajor(a):
        """[KC*128, MC*128] -> [128, MC, KC, 128] (m-major slices)."""
        a = np.asarray(a)
        return np.ascontiguousarray(
            a.reshape(KC, 128, MC, 128).transpose(1, 2, 0, 3)
        ).astype(np.float16)

    in_maps = []
    for core in range(NCORES):
        b, hg = core // 2, core % 2
        csl = slice(hg * CL, (hg + 1) * CL)
        in_maps.append(
            {
                "xh": pmajor(x[b].T.astype(np.float16), KC),
                "wq": pm_mmajor(W_attn[csl, :].T.astype(np.float16)),
                "wk": pm_mmajor(
                    W_attn[C + hg * CL : C + (hg + 1) * CL, :].T.astype(np.float16)
                ),
                "wv": pmajor(
                    W_attn[2 * C + hg * CL : 2 * C + (hg + 1) * CL, :].T.astype(
                        np.float16
                    ),
                    KC,
                ),
                "wp": pmajor(W_proj[:, csl].T.astype(np.float16), MC),
                "bqk": np.stack(
                    [b_attn[csl], b_attn[C + hg * CL : C + (hg + 1) * CL]]
                ).astype(np.float32),
                "bvr": np.ascontiguousarray(
                    np.broadcast_to(
                        b_attn[2 * C + hg * CL : 2 * C + (hg + 1) * CL].astype(
                            np.float32
                        ),
                        (128, CL),
                    )
                ),
                "wexp": np.ascontiguousarray(
                    wexp_all[hg * HL : (hg + 1) * HL].reshape(NPAIR, 2, 128, EA_W)
                ),
                "ident": np.eye(128, dtype=np.float16),
            }
        )
    return in_maps


def kernel(x, W_attn, b_attn, W_proj, b_proj, rel_emb):
    x = np.asarray(x)
    W_attn = np.asarray(W_attn)
    b_attn = np.asarray(b_attn)
    W_proj = np.asarray(W_proj)
    b_proj = np.asarray(b_proj)
    rel_emb = np.asarray(rel_emb)

    in_maps = _make_in_maps(x, W_attn, b_attn, W_proj, rel_emb)
    nc = _get_program()
    res = bass_utils.run_bass_kernel_spmd(nc, in_maps, core_ids=list(range(NCORES)))
    global LAST_RESULTS
    LAST_RESULTS = res

    y = np.empty((B, T, C), dtype=np.float32)
    for b in range(B):
        ypT = res.results[2 * b]["yp"].astype(np.float32) + res.results[2 * b + 1][
            "yp"
        ].astype(np.float32)
        y[b] = ypT.T + b_proj[None, :].astype(np.float32)
    return y


# revision 54
# speedup vs baseline: 1.0229x; 1.0068x over previous
"""Trainium2 Bass kernel for causal self-attention with T5 relative position bias.

Problem (hardcoded): B=4, T=2048, C=1024, H=16, D=64, NUM_BUCKETS=32, MAX_DISTANCE=128.
Sharding over 8 cores: core c -> (batch b=c//2, head-group hg=c%2 of 8 heads).
Each core computes qkv projection for its heads, causal attention, and a partial
output projection (its heads' rows of W_proj); host sums the two partials per batch.

Key structure:
  - Heads are processed in PAIRS (2m on partitions 0-63, 2m+1 on 64-127); the two
    K=64 S-matmuls of a pair row-tile the PE at (0,0)/(64,0).
  - AV is "flipped": P tiles [tk,128tq] stationary, v+ones [tk,65] moving ->
    out [tq,65]; softmax rowsum lands as a per-partition column so normalization
    is a DVE reciprocal + tensor_scalar multiply.
  - The T5 bias table is host-divided by exp(b31); far tiles skip the mask-mult;
    exp uses a 2-level AP that skips head-B's dead zone on diagonal tiles.
  - Pipeline shape (measured-bottleneck driven; ~258us vs 287us baseline):
    * Inputs are host-pre-arranged partition-major so DMA descriptors are
      >=2KB; loads are priority-ordered across the sync+scalar HW-DGE queues
      (m0 weight slices + x first-quarters first) so the first S matmul lands
      ~28us in; warm-up dummy matmuls keep the PE HAM clock at 2.4 GHz.
    * Within a pair, j (k-tile index) runs DESCENDING (diagonal first); the
      last j's AV + normalize + transposes are DEFERRED into the next pair's
      stream, right after its first S pair, so the PE queue never head-blocks
      on exp at pair boundaries and the ACT engine stays saturated.
    * Fillers run BEFORE each S pair: chunk0: rest of qkv0 + qkv1; chunk1:
      qkv2; chunk2: qkv3; chunk3: proj0-2 (the ACT-bound chunk absorbs proj).
      In chunk 3 pops are gated off the first iterations of each pair to dodge
      a false DMA-counter dependency on the freshly issued pair-tail
      transposes (the HWDGE completion semaphore counts in queue order).
    * yp stores go on the gpsimd SWDGE queue (sharing the sync queue with the
      xbar transposes produced wrong results on HW); the final chunk's stores
      use the scalar queue, free once the last exp is done.
    * Tail: the last pair's y is transposed on the PE (identity matmul), so
      proj(chunk3) has no DMA dependencies and overlaps the tail drain.
"""

import sys

sys.path.insert(0, "/opt/trn_rl_repo")

import math
from collections import deque

import numpy as np

import concourse.bacc as bacc
import concourse.bass as bass
import concourse.mybir as mybir
import concourse.tile as tile
from concourse import bass_utils


def _ensure_axon_hooks():
    """bass_utils imports antenv.axon_hooks when BASS_TRACE is set under axon;
    this image's antenv lacks that submodule. Provide an inert one so a stray
    trace env var degrades to a warning instead of crashing the run."""
    try:
        import antenv.axon_hooks  # noqa: F401
    except Exception:
        try:
            import types

            import antenv

            hooks = types.ModuleType("antenv.axon_hooks")
            hooks._hook = None
            hooks.set_axon_ntff_profile_hook = lambda h: setattr(hooks, "_hook", h)
            hooks.get_axon_ntff_profile_hook = lambda: hooks._hook
            sys.modules["antenv.axon_hooks"] = hooks
            antenv.axon_hooks = hooks
        except Exception:
            pass


_ensure_axon_hooks()

B, T, C = 4, 2048, 1024
H, D = 16, 64
NUM_BUCKETS, MAX_DISTANCE = 32, 128
HL = 8  # local heads per core
CL = HL * D  # 512 local channels
NCORES = 8
NPAIR = HL // 2  # 4 head pairs per core

FP16 = mybir.dt.float16
FP32 = mybir.dt.float32

NT = T // 512  # 4 tq chunks of 512
NK = T // 128  # 16 tk tiles of 128
KC = C // 128  # 8 contraction chunks for qkv
MC = CL // 128  # 4 m-chunks of local channels

# ea table geometry: slice start s = (tq0 - tk0) + 384; near tiles only, so
# s in {0,128,256,384,512} and max used column is s+511 = 1023.
EA_W = 1024


def _build_program():
    nc = bacc.Bacc(None, target_bir_lowering=False)

    # Inputs are host-pre-arranged partition-major ([128, kc, ...]) so every
    # DMA descriptor covers >=2KB contiguous per partition row.  wq/wk are
    # additionally m-major so the m=0 slices (the only ones the startup
    # q/k chains need) can be prioritized on the load queues.
    xh = nc.dram_tensor("xh", [128, KC, T], FP16, kind="ExternalInput")
    wq = nc.dram_tensor("wq", [128, MC, KC, 128], FP16, kind="ExternalInput")
    wk = nc.dram_tensor("wk", [128, MC, KC, 128], FP16, kind="ExternalInput")
    wv = nc.dram_tensor("wv", [128, KC, CL], FP16, kind="ExternalInput")
    wp = nc.dram_tensor("wp", [128, MC, C], FP16, kind="ExternalInput")
    bqk = nc.dram_tensor("bqk", [2, CL], FP32, kind="ExternalInput")
    bvr = nc.dram_tensor("bvr", [128, CL], FP32, kind="ExternalInput")
    # per-PAIR tables: [pair, head-in-pair, 128, EA_W], host-divided by exp(b31)
    wexp = nc.dram_tensor("wexp", [NPAIR, 2, 128, EA_W], FP16, kind="ExternalInput")
    ident = nc.dram_tensor("ident", [128, 128], FP16, kind="ExternalInput")
    yp = nc.dram_tensor("yp", [C, T], FP16, kind="ExternalOutput")

    with tile.TileContext(nc) as tc:
        with (
            tc.tile_pool(name="w", bufs=1) as wpool,
            tc.tile_pool(name="big", bufs=1) as bigpool,
            tc.tile_pool(name="ea", bufs=1) as eapool,
            tc.tile_pool(name="p2", bufs=6) as p2pool,
            tc.tile_pool(name="pm", bufs=6) as pmpool,
            tc.tile_pool(name="sm", bufs=2) as smpool,
            tc.tile_pool(name="yo", bufs=6) as yopool,
            tc.tile_pool(name="ps", bufs=2, space="PSUM") as ps,
            tc.tile_pool(name="pav", bufs=2, space="PSUM") as pavp,
            tc.tile_pool(name="misc", bufs=2, space="PSUM") as miscp,
        ):
            # ---- weights / constants ----
            wq_sb = wpool.tile([128, KC, CL], FP16)
            wk_sb = wpool.tile([128, KC, CL], FP16)
            wv_sb = wpool.tile([128, KC, CL], FP16)
            wp_sb = wpool.tile([128, MC, C], FP16)
            bq_sb = wpool.tile([128, MC], FP32)
            bk_sb = wpool.tile([128, MC], FP32)
            bv_sb = wpool.tile([128, CL], FP32)
            xt_sb = bigpool.tile([128, KC, T], FP16)
            ea_sb = [
                eapool.tile([128, 2, EA_W], FP16, name=f"ea{p}") for p in range(NPAIR)
            ]

            bqk_r = bqk.rearrange("b (m p) -> b p m", p=128)
            wexp_r = wexp.rearrange("pr h p w -> pr p h w")

            # Priority-ordered loads split over the two HW-DGE queues
            # (sync + scalar), most-critical bytes first: the m=0 q/k weight
            # slices, then x (split across both queues), then wv/ea0 (first
            # pair's AV + mask), then the remaining weight slices.
            id_sb = wpool.tile([128, 128], FP16)

            nc.sync.dma_start(out=bq_sb, in_=bqk_r[0])
            nc.sync.dma_start(out=bk_sb, in_=bqk_r[1])
            nc.sync.dma_start(out=wq_sb[:, :, 0:128], in_=wq[:, 0])
            for kc in range(4):
                nc.sync.dma_start(out=xt_sb[:, kc, 0:512], in_=xh[:, kc, 0:512])
            nc.sync.dma_start(out=wv_sb, in_=wv[:])
            nc.sync.dma_start(out=bv_sb, in_=bvr[:])
            for m in range(1, MC):
                nc.sync.dma_start(
                    out=wq_sb[:, :, m * 128 : (m + 1) * 128], in_=wq[:, m]
                )
            for kc in range(4):
                nc.sync.dma_start(out=xt_sb[:, kc, 512:1024], in_=xh[:, kc, 512:1024])
            for kc in range(KC):
                nc.sync.dma_start(
                    out=xt_sb[:, kc, 1024:2048], in_=xh[:, kc, 1024:2048]
                )
            nc.sync.dma_start(out=wp_sb, in_=wp[:])
            nc.sync.dma_start(out=id_sb, in_=ident[:])

            nc.scalar.dma_start(out=wk_sb[:, :, 0:128], in_=wk[:, 0])
            for kc in range(4, KC):
                nc.scalar.dma_start(out=xt_sb[:, kc, 0:512], in_=xh[:, kc, 0:512])
            for kc in range(4, KC):
                nc.scalar.dma_start(out=xt_sb[:, kc, 512:1024], in_=xh[:, kc, 512:1024])
            nc.scalar.dma_start(out=ea_sb[0], in_=wexp_r[0])
            for m in range(1, MC):
                nc.scalar.dma_start(
                    out=wk_sb[:, :, m * 128 : (m + 1) * 128], in_=wk[:, m]
                )
                nc.scalar.dma_start(out=ea_sb[m], in_=wexp_r[m])

            # ---- persistent activations ----
            qT_sb = bigpool.tile([128, MC, T], FP16)  # c' = m*128 + p
            kT_sb = bigpool.tile([128, MC, T], FP16)
            v_sb = bigpool.tile([128, NK, HL * 65], FP16)  # slot l: [v(64), ones]
            y_sb = bigpool.tile([128, MC, T], FP16)  # y_cat_T, c_in = m*128 + p

            for l in range(HL):
                nc.vector.memset(v_sb[:, :, l * 65 + 64 : l * 65 + 65], 1.0)

            # Preload the exp activation table before any real work
            warm = smpool.tile([1, 2], FP32, tag="warm")
            nc.vector.memset(warm[:], 0.0)
            warm2 = smpool.tile([1, 2], FP16, tag="warm2")
            nc.scalar.activation(
                out=warm2[:], in_=warm[:],
                func=mybir.ActivationFunctionType.Exp, scale=1.0,
            )

            # ---- qkv / proj closures (PE fillers during attention) ----
            def qk_closure(tch, m, w_sb, b_sb, out_sb):
                def emit():
                    tsl = slice(tch * 512, (tch + 1) * 512)
                    msl = slice(m * 128, (m + 1) * 128)
                    pq = miscp.tile([128, 512], FP32, tag="misc")
                    for kc in range(KC):
                        nc.tensor.matmul(
                            pq[:],
                            w_sb[:, kc, msl],
                            xt_sb[:, kc, tsl],
                            start=(kc == 0),
                            stop=(kc == KC - 1),
                        )
                    nc.vector.tensor_scalar_add(
                        out=out_sb[:, m, tsl], in0=pq[:], scalar1=b_sb[:, m : m + 1]
                    )

                return emit

            def v_closure(tch, ts):
                def emit():
                    t16 = tch * 4 + ts
                    pv = miscp.tile([128, 512], FP32, tag="misc")
                    for kc in range(KC):
                        nc.tensor.matmul(
                            pv[:],
                            xt_sb[:, kc, t16 * 128 : (t16 + 1) * 128],
                            wv_sb[:, kc, :],
                            start=(kc == 0),
                            stop=(kc == KC - 1),
                        )
                    # scatter into 65-wide slots (even/odd strided copies) + bias
                    for par in range(2):
                        src = bass.AP(
                            tensor=pv.tensor,
                            offset=pv.offset + par * 64,
                            ap=[pv.ap[0], [128, 4], [1, 64]],
                        )
                        srcb = bass.AP(
                            tensor=bv_sb.tensor,
                            offset=bv_sb.offset + par * 64,
                            ap=[bv_sb.ap[0], [128, 4], [1, 64]],
                        )
                        base = v_sb[:, t16]
                        dst = bass.AP(
                            tensor=base.tensor,
                            offset=base.offset + par * 65,
                            ap=[base.ap[0], [130, 4], [1, 64]],
                        )
                        nc.vector.tensor_add(out=dst, in0=src, in1=srcb)

                return emit

            def proj_closure(tch, mo):
                def emit():
                    tsl = slice(tch * 512, (tch + 1) * 512)
                    osl = slice(mo * 128, (mo + 1) * 128)
                    pp = miscp.tile([128, 512], FP32, tag="misc")
                    for kcm in range(MC):
                        nc.tensor.matmul(
                            pp[:],
                            wp_sb[:, kcm, osl],
                            y_sb[:, kcm, tsl],
                            start=(kcm == 0),
                            stop=(kcm == MC - 1),
                        )
                    yo_sb = yopool.tile([128, 512], FP16, tag="yo")
                    nc.vector.tensor_copy(yo_sb[:], pp[:])
                    # stores on the (otherwise idle) gpsimd SWDGE queue: on the
                    # sync HWDGE queue they can overlap in flight with the
                    # xbar transposes and produced wrong results on HW.  The
                    # final chunk's stores go on the scalar HWDGE queue (ACT
                    # is finished by then) so the tail isn't SWDGE-drain-bound.
                    if tch == NT - 1:
                        nc.scalar.dma_start(out=yp[osl, tsl], in_=yo_sb[:])
                    else:
                        nc.gpsimd.dma_start(out=yp[osl, tsl], in_=yo_sb[:])

                return emit

            def warm_pe(n):
                # tiny no-dep matmuls keep the HAM activity window busy so
                # surrounding real matmuls run at 2.4 GHz instead of 1.2
                wps = miscp.tile([1, 16], FP32, tag="misc")
                ones = v_sb[0:64, 0, 64:65]
                for i in range(n):
                    nc.tensor.matmul(
                        wps[0:1, i % 16 : i % 16 + 1],
                        ones,
                        ones,
                        start=True,
                        stop=True,
                        skip_group_check=True,
                    )

            # ---- startup: minimal qkv before attention ----
            warm_pe(170)
            qk_closure(0, 0, wq_sb, bq_sb, qT_sb)()
            qk_closure(0, 0, wk_sb, bk_sb, kT_sb)()
            for ts in (3, 2):
                v_closure(0, ts)()

            # ---- filler schedule ----
            filler_lists = {
                0: (
                    [v_closure(0, 1), v_closure(0, 0)]
                    + [qk_closure(0, m, wq_sb, bq_sb, qT_sb) for m in (1,)]
                    + [qk_closure(0, m, wk_sb, bk_sb, kT_sb) for m in (1,)]
                    + [qk_closure(0, m, wq_sb, bq_sb, qT_sb) for m in (2,)]
                    + [qk_closure(0, m, wk_sb, bk_sb, kT_sb) for m in (2,)]
                    + [qk_closure(0, m, wq_sb, bq_sb, qT_sb) for m in (3,)]
                    + [qk_closure(0, m, wk_sb, bk_sb, kT_sb) for m in (3,)]
                    + [v_closure(1, ts) for ts in (3, 2, 1, 0)]
                    + [
                        cl
                        for m in range(MC)
                        for cl in (
                            qk_closure(1, m, wq_sb, bq_sb, qT_sb),
                            qk_closure(1, m, wk_sb, bk_sb, kT_sb),
                        )
                    ]
                ),
                1: (
                    [
                        cl
                        for m in range(2)
                        for cl in (
                            qk_closure(2, m, wq_sb, bq_sb, qT_sb),
                            qk_closure(2, m, wk_sb, bk_sb, kT_sb),
                        )
                    ]
                    + [v_closure(2, ts) for ts in (3, 2)]
                    + [
                        cl
                        for m in range(2, MC)
                        for cl in (
                            qk_closure(2, m, wq_sb, bq_sb, qT_sb),
                            qk_closure(2, m, wk_sb, bk_sb, kT_sb),
                        )
                    ]
                    + [v_closure(2, ts) for ts in (1, 0)]
                ),
                2: (
                    [
                        cl
                        for m in range(2)
                        for cl in (
                            qk_closure(3, m, wq_sb, bq_sb, qT_sb),
                            qk_closure(3, m, wk_sb, bk_sb, kT_sb),
                        )
                    ]
                    + [v_closure(3, ts) for ts in (3, 2)]
                    + [
                        cl
                        for m in range(2, MC)
                        for cl in (
                            qk_closure(3, m, wq_sb, bq_sb, qT_sb),
                            qk_closure(3, m, wk_sb, bk_sb, kT_sb),
                        )
                    ]
                    + [v_closure(3, ts) for ts in (1, 0)]
                ),
                3: (
                    [proj_closure(0, mo) for mo in range(C // 128)]
                    + [proj_closure(1, mo) for mo in range(C // 128)]
                    + [proj_closure(2, mo) for mo in range(C // 128)]
                ),
            }

            # ---- attention emission: descending j, deferred pair tails ----
            def make_tail(pavA, pavB, deferred_avs, pair, c, final=False):
                def emit():
                    for av in deferred_avs:
                        av()
                    if final:
                        warm_pe(40)
                    rec = smpool.tile([128, 8], FP32, tag="rec")
                    recA = bass.AP(
                        tensor=pavA.tensor,
                        offset=pavA.offset + 64,
                        ap=[pavA.ap[0], [65, 4]],
                    )
                    recB = bass.AP(
                        tensor=pavB.tensor,
                        offset=pavB.offset + 64,
                        ap=[pavB.ap[0], [65, 4]],
                    )
                    nc.vector.reciprocal(out=rec[:, 0:4], in_=recA)
                    nc.vector.reciprocal(out=rec[:, 4:8], in_=recB)
                    yn = smpool.tile([128, 4, 128], FP16, tag="yn")
                    for k in range(4):
                        nc.vector.tensor_scalar_mul(
                            out=yn[:, k, 0:64],
                            in0=pavA[:, k * 65 : k * 65 + 64],
                            scalar1=rec[:, k : k + 1],
                        )
                        nc.vector.tensor_scalar_mul(
                            out=yn[:, k, 64:128],
                            in0=pavB[:, k * 65 : k * 65 + 64],
                            scalar1=rec[:, 4 + k : 5 + k],
                        )
                    for k in range(4):
                        ysl = y_sb[:, pair, c * 512 + k * 128 : c * 512 + (k + 1) * 128]
                        if final:
                            # last pair: PE transpose + DVE copy, so the
                            # trailing projection has no DMA-queue deps
                            pt = pavp.tile([128, 128], FP16, tag="pav")
                            nc.tensor.transpose(out=pt[:], in_=yn[:, k], identity=id_sb)
                            nc.vector.tensor_copy(ysl, pt[:])
                        else:
                            # SBUF->SBUF xbar DMA transpose [tq,128] -> [c',tq]
                            nc.sync.dma_start(out=ysl, in_=yn[:, k], transpose=True)

                return emit

            pending_tail = None

            for c in range(NT):
                fillers = deque(filler_lists[c])
                nj = 4 * c + 4
                total_iters = NPAIR * nj
                nfill = len(fillers)
                it = 0
                popped = 0

                for pair in range(NPAIR):
                    pavA = pavp.tile([128, 512], FP32, tag="pav")
                    pavB = pavp.tile([128, 512], FP32, tag="pav")
                    deferred = None
                    for idx, j in enumerate(range(nj - 1, -1, -1)):
                        off = max(0, 128 * j - 512 * c)  # multiple of 128
                        far = j <= 4 * c - 2
                        s = 512 * c - 128 * j + 384

                        # PE fillers BEFORE the S pair: when S is ACT-paced,
                        # the fillers run during its wait instead of queuing
                        # behind it.  In chunk 3 the fillers are proj closures
                        # whose DMA-queue semaphore targets count the freshly
                        # issued pair-tail transposes (false dependency); hold
                        # pops until idx>=4 so those have drained.
                        # (chunk 3 fillers drain 8 iters early so their CASTs
                        # clear the DVE queue before the final pair's tail;
                        # pops are gated off idx<=4 — the pair-tail transposes
                        # issue late behind the DVE and take ~7us to drain —
                        # and off the chunk entry so the first S isn't delayed)
                        it += 1
                        lead = 8 if c == 3 else 0
                        due = nfill * min(it + lead, total_iters) // total_iters
                        gated = c == 3 and (1 <= idx <= 4 or (pair == 0 and idx == 0))
                        if not gated:
                            while popped < due:
                                fillers.popleft()()
                                popped += 1

                        pS = ps.tile([128, 1024], FP32, tag="pS")
                        nc.tensor.matmul(
                            pS[:, off:512],
                            kT_sb[0:64, pair, j * 128 : (j + 1) * 128],
                            qT_sb[0:64, pair, c * 512 + off : (c + 1) * 512],
                            start=True,
                            stop=True,
                        )
                        nc.tensor.matmul(
                            pS[:, 512 + off : 1024],
                            kT_sb[64:128, pair, j * 128 : (j + 1) * 128],
                            qT_sb[64:128, pair, c * 512 + off : (c + 1) * 512],
                            start=True,
                            stop=True,
                        )

                        # previous pair's deferred tail: after this pair's
                        # first S so the PE queue never head-blocks on exp
                        if idx == 0 and pending_tail is not None:
                            pending_tail()
                            pending_tail = None

                        p2 = p2pool.tile([128, 1024], FP16, tag="p2")
                        # 2-level AP skips head-B's dead zone [512:512+off]
                        # (never written by S; saves ACT cycles on diag tiles)
                        exp_in = bass.AP(
                            tensor=pS.tensor,
                            offset=pS.offset + off,
                            ap=[pS.ap[0], [512, 2], [1, 512 - off]],
                        )
                        exp_out = bass.AP(
                            tensor=p2.tensor,
                            offset=p2.offset + off,
                            ap=[p2.ap[0], [512, 2], [1, 512 - off]],
                        )
                        nc.scalar.activation(
                            out=exp_out,
                            in_=exp_in,
                            func=mybir.ActivationFunctionType.Exp,
                            scale=1.0 / math.sqrt(D),
                        )
                        if far:
                            pmt = p2
                        else:
                            pmt = pmpool.tile([128, 1024], FP16, tag="pm")
                            ea_pair = bass.AP(
                                tensor=ea_sb[pair].tensor,
                                offset=ea_sb[pair].offset + s + off,
                                ap=[ea_sb[pair].ap[0], [EA_W, 2], [1, 512 - off]],
                            )
                            dst = bass.AP(
                                tensor=pmt.tensor,
                                offset=pmt.offset + off,
                                ap=[pmt.ap[0], [512, 2], [1, 512 - off]],
                            )
                            srcp = bass.AP(
                                tensor=p2.tensor,
                                offset=p2.offset + off,
                                ap=[p2.ap[0], [512, 2], [1, 512 - off]],
                            )
                            # (masks stay on DVE: GPSIMD tensor_mul measured
                            # ~2x slower and serializes behind the yp stores)
                            nc.vector.tensor_mul(out=dst, in0=srcp, in1=ea_pair)

                        def av_emit(
                            pmt=pmt, j=j, off=off, idx=idx, pavA=pavA, pavB=pavB, pair=pair
                        ):
                            # start=True clears the has_written map of the
                            # WHOLE PSUM bank; with descending j the first
                            # emitted AV (j=nj-1, k=3) sets it and every later
                            # matmul overwrites-where-clear / accumulates.
                            # stop closes each subtile's group at its true
                            # last contribution (j == 0; sim-only).
                            for k in range(off // 128, 4):
                                nc.tensor.matmul(
                                    pavA[:, k * 65 : k * 65 + 65],
                                    pmt[:, k * 128 : (k + 1) * 128],
                                    v_sb[:, j, 2 * pair * 65 : 2 * pair * 65 + 65],
                                    start=(idx == 0),
                                    stop=(j == 0),
                                    skip_group_check=True,
                                )
                                nc.tensor.matmul(
                                    pavB[:, k * 65 : k * 65 + 65],
                                    pmt[:, 512 + k * 128 : 512 + (k + 1) * 128],
                                    v_sb[
                                        :, j, (2 * pair + 1) * 65 : (2 * pair + 1) * 65 + 65
                                    ],
                                    start=(idx == 0),
                                    stop=(j == 0),
                                    skip_group_check=True,
                                )

                        # the very last pair has no successor to pipeline
                        # into, so emit its final AV inline
                        if idx < nj - 1 or (c == NT - 1 and pair == NPAIR - 1):
                            av_emit()
                            deferred = deferred or []
                        else:
                            deferred = [av_emit]

                    pending_tail = make_tail(
                        pavA, pavB, deferred, pair, c,
                        final=(c == NT - 1 and pair == NPAIR - 1),
                    )

                while fillers:
                    fillers.popleft()()

            # final pair tail + last chunk's projection (kcm 0-2 of each mo
            # overlap the tail transposes; only kcm=3 waits on them)
            pending_tail()
            for mo in range(C // 128):
                proj_closure(NT - 1, mo)()

    nc.compile()
    return nc


_NC = None
LAST_RESULTS = None


def _get_program():
    global _NC
    if _NC is None:
        _NC = _build_program()
    return _NC


# Bucket b covers distances d in [starts[b], starts[b+1]); verified bit-exact
# against the jax reference's _relative_position_bucket for T=2048.
_BUCKET_STARTS = np.array(
    [0, 1, 2, 3, 4, 5, 6, 7, 8, 9, 10, 11, 12, 13, 14, 15,
     16, 18, 20, 23, 26, 29, 33, 38, 43, 49, 55, 63, 72, 82, 93, 106]
)


def _rel_bias_buckets():
    """bucket(d) for d = q - k in [0, T)."""
    d = np.arange(T)
    return np.searchsorted(_BUCKET_STARTS, d, side="right") - 1


def _make_in_maps(x, W_attn, b_attn, W_proj, rel_emb):
    buckets = _rel_bias_buckets()  # [T]
    bias_by_dist = rel_emb[buckets, :]  # [T, H] fp32
    # Divide by exp(b31) per head: far tiles (all d >= 106, bucket 31) then
    # multiply by exactly 1.0 and can skip the mask-multiply; the factor
    # cancels in the softmax ratio.
    b31 = rel_emb[NUM_BUCKETS - 1, :]  # [H]
    # vec[h, j] = exp(bias[j - 511] - b31[h]) for j >= 511 else 0.  Only
    # distances d < EA_W - 384 ever appear in near tiles (d = s + tq - p - 384
    # with s <= 512), so the table is clipped to EA_W + 127 columns.
    ndist = EA_W + 127 - 511  # 640
    vec = np.zeros((H, EA_W + 127), dtype=np.float32)
    vec[:, 511:] = np.exp(bias_by_dist[:ndist].T - b31[:, None])
    vec = vec.astype(np.float16)
    # expand to the per-head Toeplitz table A[h, p, x] = vec[h, x - p + 127]
    sw = np.lib.stride_tricks.sliding_window_view(vec, EA_W, axis=1)  # [H,128,EA_W]
    wexp_all = np.ascontiguousarray(sw[:, ::-1, :])  # [H, 128, EA_W]

    def pmajor(a, nblk):
        """[nblk*128, M] -> [128, nblk, M] partition-major contiguous fp16."""
        a = np.asarray(a)
        return np.ascontiguousarray(
            a.reshape(nblk, 128, a.shape[1]).transpose(1, 0, 2)
        ).astype(np.float16)

    def pm_mmajor(a):
        """[KC*128, MC*128] -> [128, MC, KC, 128] (m-major slices)."""
        a = np.asarray(a)
        return np.ascontiguousarray(
            a.reshape(KC, 128, MC, 128).transpose(1, 2, 0, 3)
        ).astype(np.float16)

    in_maps = []
    for core in range(NCORES):
        b, hg = core // 2, core % 2
        csl = slice(hg * CL, (hg + 1) * CL)
        in_maps.append(
            {
                "xh": pmajor(x[b].T.astype(np.float16), KC),
                "wq": pm_mmajor(W_attn[csl, :].T.astype(np.float16)),
                "wk": pm_mmajor(
                    W_attn[C + hg * CL : C + (hg + 1) * CL, :].T.astype(np.float16)
                ),
                "wv": pmajor(
                    W_attn[2 * C + hg * CL : 2 * C + (hg + 1) * CL, :].T.astype(
                        np.float16
                    ),
                    KC,
                ),
                "wp": pmajor(W_proj[:, csl].T.astype(np.float16), MC),
                "bqk": np.stack(
                    [b_attn[csl], b_attn[C + hg * CL : C + (hg + 1) * CL]]
                ).astype(np.float32),
                "bvr": np.ascontiguousarray(
                    np.broadcast_to(
                        b_attn[2 * C + hg * CL : 2 * C + (hg + 1) * CL].astype(
                            np.float32
                        ),
                        (128, CL),
                    )
                ),
                "wexp": np.ascontiguousarray(
                    wexp_all[hg * HL : (hg + 1) * HL].reshape(NPAIR, 2, 128, EA_W)
                ),
                "ident": np.eye(128, dtype=np.float16),
            }
        )
    return in_maps


def kernel(x, W_attn, b_attn, W_proj, b_proj, rel_emb):
    x = np.asarray(x)
    W_attn = np.asarray(W_attn)
    b_attn = np.asarray(b_attn)
    W_proj = np.asarray(W_proj)
    b_proj = np.asarray(b_proj)
    rel_emb = np.asarray(rel_emb)

    in_maps = _make_in_maps(x, W_attn, b_attn, W_proj, rel_emb)
    nc = _get_program()
    res = bass_utils.run_bass_kernel_spmd(nc, in_maps, core_ids=list(range(NCORES)))
    global LAST_RESULTS
    LAST_RESULTS = res

    y = np.empty((B, T, C), dtype=np.float32)
    for b in range(B):
        ypT = res.results[2 * b]["yp"].astype(np.float32) + res.results[2 * b + 1][
            "yp"
        ].astype(np.float32)
        y[b] = ypT.T + b_proj[None, :].astype(np.float32)
    return y
